# revision 2
# baseline (speedup 1.0000x reference)
"""DenseGATConv Trainium2 kernel (8 NeuronCores, SPMD, column-sharded).

Math (identical to baseline):
    m[i,j] = adj[i,j] * max(exp(a_src_i) * q_j, u_i),  q_j = exp(0.8 a_dst_j),
    u_i = exp(0.2 a_src_i);  out[j,:] = (m^T h)[j,:] / colsum(m)[j] + bias
(the per-column factor exp(0.2 a_dst_j) cancels between num and den).

Structure (measured on HW, see session notes):
  * 129-col matmul pattern: lhsT = mask 128-col slice (stationary), rhs =
    H'_t = [h_t | a_src_t | ones] (moving, 130 cols). The 8 j-chunk PSUM
    accumulators produce numerator AND denominator (col 129) in one stream:
    34.7us vs 101us for the classic num+den pattern.
  * One Act copy per i-tile moves [h | a_src] PSUM->SBUF; exp(a_src) is read
    back with a stride-CW access pattern (groups of GRP columns).
  * h tiles are computed one adj-chunk ahead, interleaved tile-by-tile.
  * The rep loop (For_i) body is emitted UNROLL times with ping-pong
    per-iteration tiles (hbig/q_rep/W/...) so consecutive reps pipeline
    instead of barriering on write-after-read hazards.
"""

import numpy as np
from contextlib import ExitStack

import concourse.bass as bass
import concourse.bacc as bacc
import concourse.tile as tile
from concourse import mybir
from concourse.bass_utils import run_bass_kernel_spmd

F32 = mybir.dt.float32
F16 = mybir.dt.float16
ALU = mybir.AluOpType
ACTF = mybir.ActivationFunctionType

N, C_IN, C_OUT = 8192, 256, 128
NCORES = 8
JB = N // NCORES          # 1024 destination columns per core
NT = N // 128             # 64 i-tiles
CHUNK = 8                 # i-tiles per adj DMA
NCH = NT // CHUNK
GRP = 4                   # a_src exp-group size (i-tiles)
NJC = JB // 128           # 8 j-chunks of 128 columns
CW = 130                  # rhs width: 128 h + a_src col + ones col
XB = 16                   # i-tiles per xT chunk
UNROLL = 1                # rep-loop unroll (1: measured best)
ADJ_QUARTER = False       # ablation: DMA only 1/4 of each adj chunk

_nc_cache = {}


def _make_pools(tc, ctx):
    p = {}
    p["it2"] = ctx.enter_context(tc.tile_pool(name="it2", bufs=UNROLL))
    p["xt"] = ctx.enter_context(tc.tile_pool(name="xt", bufs=4))
    p["scr"] = ctx.enter_context(tc.tile_pool(name="scr", bufs=2))
    p["adj"] = ctx.enter_context(tc.tile_pool(name="adj", bufs=4))
    p["t2"] = ctx.enter_context(tc.tile_pool(name="t2", bufs=6))
    p["m"] = ctx.enter_context(tc.tile_pool(name="m", bufs=6))
    p["psacc"] = ctx.enter_context(
        tc.tile_pool(name="psacc", bufs=1, space="PSUM"))
    p["ctx"] = ctx
    return p


def _emit_body(tc, nc, p, tensors, sfx):
    (xT_in, xTloc_in, adj_in, W_in, asrc_rep_in, adst_rep_in, out_nd) = tensors

    adj_r = adj_in.rearrange("(c a p) j -> c p a j", a=CHUNK, p=128)
    it2, xt_pool, scratch = p["it2"], p["xt"], p["scr"]
    adj_pool, t2_pool, m_pool = p["adj"], p["t2"], p["m"]
    ps_acc = p["psacc"]

    # ---- constants -------------------------------------------------
    W_sb = it2.tile([128, C_IN], F16, tag="W_sb", name=f"W_sb{sfx}")
    nc.sync.dma_start(W_sb[:, 0:128], W_in[:, 0:128])
    nc.sync.dma_start(W_sb[:, 128:256], W_in[:, 128:256])
    attsrc = it2.tile([128, C_OUT], F32, tag="attsrc", name=f"attsrc{sfx}")
    nc.sync.dma_start(attsrc[:], asrc_rep_in[:])
    attdst = it2.tile([128, C_OUT], F32, tag="attdst", name=f"attdst{sfx}")
    nc.sync.dma_start(attdst[:], adst_rep_in[:])
    ones_row = it2.tile([1, 128], F16, tag="ones_row", name=f"ones_row{sfx}")
    nc.vector.memset(ones_row[:], 1.0)

    # ---- a_dst path -> q_rep --------------------------------------
    wdst = it2.tile([128, 2], F32, tag="wdst", name=f"wdst{sfx}")
    for k in range(2):
        sc = scratch.tile([128, C_OUT], F32, tag="scr", name=f"scw{k}{sfx}")
        nc.vector.scalar_tensor_tensor(
            sc[:], W_sb[:, k * 128:(k + 1) * 128], 1.0, attdst[:],
            op0=ALU.mult, op1=ALU.mult, accum_out=wdst[:, k:k + 1],
        )
    wdst_h = it2.tile([128, 2], F16, tag="wdst_h", name=f"wdst_h{sfx}")
    nc.vector.tensor_copy(wdst_h[:], wdst[:])
    # w_src column (fused a_src matmul column inside Wm)
    wsrc = it2.tile([128, 2], F32, tag="wsrc", name=f"wsrc{sfx}")
    for k in range(2):
        sc = scratch.tile([128, C_OUT], F32, tag="scr", name=f"scs{k}{sfx}")
        nc.vector.scalar_tensor_tensor(
            sc[:], W_sb[:, k * 128:(k + 1) * 128], 1.0, attsrc[:],
            op0=ALU.mult, op1=ALU.mult, accum_out=wsrc[:, k:k + 1],
        )
    # merged rhs for the h matmul: [W_k (128) | wsrc_k (1)] per k-block
    Wm = it2.tile([128, 2 * 129], F16, tag="Wm", name=f"Wm{sfx}")
    for k in range(2):
        nc.scalar.copy(Wm[:, k * 129:k * 129 + 128],
                       W_sb[:, k * 128:(k + 1) * 128])
        nc.vector.tensor_copy(Wm[:, k * 129 + 128:k * 129 + 129],
                              wsrc[:, k:k + 1])
    # a_dst row over the local block, then broadcast+exp -> q_rep
    adst_row = it2.tile([1, JB], F32, tag="adst_row", name=f"adst_row{sfx}")
    adst_row_h = it2.tile([1, JB], F16, tag="adst_rh", name=f"adst_rh{sfx}")
    q_rep = it2.tile([128, JB], F16, tag="q_rep", name=f"q_rep{sfx}")
    with tc.tile_pool(name=f"pspre{sfx}", bufs=1, space="PSUM") as ps_pre:
        for jb in range(JB // 512):
            ap = ps_pre.tile([1, 512], F32, tag="adst", name=f"adstp{jb}{sfx}")
            for k in range(2):
                xl = xt_pool.tile([128, 512], F16, tag="xtloc",
                                  name=f"xl{jb}_{k}{sfx}")
                nc.sync.dma_start(
                    xl[:], xTloc_in[k * 128:(k + 1) * 128,
                                    jb * 512:(jb + 1) * 512])
                nc.tensor.matmul(ap[:], lhsT=wdst_h[:, k:k + 1],
                                 rhs=xl[:], start=(k == 0), stop=(k == 1))
            nc.scalar.copy(adst_row[0:1, jb * 512:(jb + 1) * 512], ap[:])
            nc.vector.tensor_copy(adst_row_h[0:1, jb * 512:(jb + 1) * 512],
                                  adst_row[0:1, jb * 512:(jb + 1) * 512])
            qp = ps_pre.tile([128, 512], F32, tag="qrep", name=f"qp{jb}{sfx}")
            nc.tensor.matmul(
                qp[:], lhsT=ones_row[:],
                rhs=adst_row_h[0:1, jb * 512:(jb + 1) * 512],
                start=True, stop=True)
            nc.scalar.activation(q_rep[:, jb * 512:(jb + 1) * 512], qp[:],
                                 ACTF.Exp, scale=0.8)
    ps_h = p["ctx"].enter_context(
        tc.tile_pool(name=f"psh{sfx}", bufs=4, space="PSUM"))

    # ---- persistent H' = [h | a_src | 1] big tile ------------------
    hbig = it2.tile([128, NT * CW], F16, tag="hbig", name=f"hbig{sfx}")
    nc.vector.memset(hbig[:, C_OUT + 1::CW], 1.0)   # ones cols
    ea_g = [it2.tile([128, GRP], F32, tag=f"ea{g}", name=f"ea{g}{sfx}")
            for g in range(NT // GRP)]   # exp(a_src)
    u_g = [it2.tile([128, GRP], F32, tag=f"u{g}", name=f"u{g}{sfx}")
           for g in range(NT // GRP)]    # exp(0.2 a_src)

    # ---- PSUM accumulators: 2 j-chunk regions per 2KB bank ---------
    accb = [ps_acc.tile([128, 512], F32, tag=f"acc{k}", name=f"acc{k}{sfx}")
            for k in range(NJC // 2)]
    acc = [accb[k // 2][:, (k % 2) * 256:(k % 2) * 256 + 256]
           for k in range(NJC)]

    # ---- main loop: h JIT (one chunk ahead) + t2/m + matmuls -------
    state = {}

    def emit_h_tile(t):
        cx, ti = divmod(t, XB)
        if ti == 0:
            xc = [xt_pool.tile([128, XB * 128], F16, tag="xtc",
                               name=f"xc{cx}_{k}{sfx}") for k in range(2)]
            state["xc"] = xc
            for k in range(2):
                nc.scalar.dma_start(
                    xc[k][:], xT_in[k * 128:(k + 1) * 128,
                                    cx * XB * 128:(cx + 1) * XB * 128])
        xc = state["xc"]
        g, gi = divmod(t, GRP)
        hp = ps_h.tile([128, 256], F32, tag="hps", name=f"hps{t}{sfx}")
        for k in range(2):
            nc.tensor.matmul(hp[:, 0:C_OUT + 1],
                             lhsT=xc[k][:, ti * 128:(ti + 1) * 128],
                             rhs=Wm[:, k * 129:(k + 1) * 129],
                             start=(k == 0), stop=(k == 1))
        nc.scalar.copy(hbig[:, t * CW:t * CW + C_OUT + 1],
                       hp[:, 0:C_OUT + 1])
        if gi == GRP - 1:
            asrc_view = hbig[:, g * GRP * CW + C_OUT:(g * GRP + GRP) * CW:CW]
            nc.scalar.activation(ea_g[g][:], asrc_view, ACTF.Exp, scale=1.0)
            nc.scalar.activation(u_g[g][:], asrc_view, ACTF.Exp, scale=0.2)

    adj_tiles = []
    for c in range(NCH):
        adj_ch = adj_pool.tile([128, CHUNK * JB], F16, tag="adj",
                               name=f"adj{c}{sfx}")
        if ADJ_QUARTER:
            nc.sync.dma_start(adj_ch[:, 0:CHUNK * JB // 4],
                              adj_r[c][:, 0:CHUNK // 4, :])
        else:
            nc.sync.dma_start(adj_ch[:], adj_r[c])
        adj_tiles.append(adj_ch)

    for a in range(CHUNK):
        emit_h_tile(a)
    for c in range(NCH):
        adj_ch = adj_tiles[c]
        for a in range(CHUNK):
            if c + 1 < NCH:
                emit_h_tile((c + 1) * CHUNK + a)
            t = c * CHUNK + a
            g, gi = divmod(t, GRP)
            t2 = t2_pool.tile([128, JB], F16, tag="t2", name=f"t2_{t}{sfx}")
            nc.vector.tensor_scalar(
                t2[:], q_rep[:], ea_g[g][:, gi:gi + 1], u_g[g][:, gi:gi + 1],
                op0=ALU.mult, op1=ALU.max)
            m = m_pool.tile([128, JB], F16, tag="m", name=f"m{t}{sfx}")
            nc.vector.tensor_tensor(
                m[:], t2[:], adj_ch[:, a * JB:(a + 1) * JB], op=ALU.mult)
            for k in range(NJC):
                # PSUM start zeroes the whole bank: only the first region
                # per bank starts; its bank-mate accumulates onto the
                # freshly zeroed region.
                nc.tensor.matmul(acc[k][:, 0:CW],
                                 lhsT=m[:, k * 128:(k + 1) * 128],
                                 rhs=hbig[:, t * CW:(t + 1) * CW],
                                 start=(t == 0 and k % 2 == 0),
                                 stop=(t == NT - 1),
                                 skip_group_check=(k % 2 == 1))

    # ---- epilogue --------------------------------------------------
    out_sb = it2.tile([128, NJC * CW], F32, tag="out_sb", name=f"out_sb{sfx}")
    for k in range(NJC):
        nc.scalar.copy(out_sb[:, k * CW:(k + 1) * CW], acc[k][:, 0:CW])
    nc.sync.dma_start(out_nd[:], out_sb[:])


def build_nc(reps=1):
    key = ("nc", reps)
    if key in _nc_cache:
        return _nc_cache[key]
    nc = bacc.Bacc("TRN2", target_bir_lowering=False, debug=False,
                   num_devices=NCORES)

    xT_in = nc.dram_tensor("xT", [C_IN, N], F16, kind="ExternalInput")
    xTloc_in = nc.dram_tensor("xTloc", [C_IN, JB], F16, kind="ExternalInput")
    adj_in = nc.dram_tensor("adjc", [N, JB], F16, kind="ExternalInput")
    W_in = nc.dram_tensor("Wt", [128, C_IN], F16, kind="ExternalInput")
    asrc_rep_in = nc.dram_tensor("attsrc_rep", [128, C_OUT], F32,
                                 kind="ExternalInput")
    adst_rep_in = nc.dram_tensor("attdst_rep", [128, C_OUT], F32,
                                 kind="ExternalInput")
    out_nd = nc.dram_tensor("outnd", [128, NJC * CW], F32,
                            kind="ExternalOutput")

    tensors = (xT_in, xTloc_in, adj_in, W_in, asrc_rep_in, adst_rep_in,
               out_nd)

    with tile.TileContext(nc) as tc:
        with ExitStack() as ctx:
            p = _make_pools(tc, ctx)
            if reps == 1:
                _emit_body(tc, nc, p, tensors, "_a")
            else:
                pairs, extra = divmod(reps, UNROLL)
                if pairs > 0:
                    with tc.For_i(0, pairs, 1, hint_engines=(
                            mybir.EngineType.PE, mybir.EngineType.DVE,
                            mybir.EngineType.Activation, mybir.EngineType.SP,
                            mybir.EngineType.Pool)):
                        for ui in range(UNROLL):
                            _emit_body(tc, nc, p, tensors, f"_u{ui}")
                for ei in range(extra):
                    _emit_body(tc, nc, p, tensors, f"_e{ei}")

    nc.compile()
    _nc_cache[key] = nc
    return nc


def make_in_maps(x, adj, W, att_src, att_dst):
    ch_np = np.float16
    xT = np.ascontiguousarray(x.T.astype(np.float32, copy=False)).astype(ch_np)
    Wt = np.ascontiguousarray(
        np.concatenate([W[0:128, :], W[128:256, :]], axis=1)).astype(ch_np)
    attsrc_rep = np.ascontiguousarray(
        np.broadcast_to(att_src.astype(np.float32), (128, C_OUT)))
    attdst_rep = np.ascontiguousarray(
        np.broadcast_to(att_dst.astype(np.float32), (128, C_OUT)))
    in_maps = []
    for d in range(NCORES):
        adj_d = np.ascontiguousarray(
            adj[:, d * JB:(d + 1) * JB].astype(np.float32, copy=False))
        idx = np.arange(JB)
        adj_d[d * JB + idx, idx] = 1.0          # self loops
        adj_d = adj_d.astype(ch_np)             # 0/1: exact
        xTloc = np.ascontiguousarray(xT[:, d * JB:(d + 1) * JB])
        in_maps.append({
            "xT": xT, "xTloc": xTloc, "adjc": adj_d, "Wt": Wt,
            "attsrc_rep": attsrc_rep, "attdst_rep": attdst_rep,
        })
    return in_maps


def postprocess(results, bias):
    blocks = []
    for d in range(NCORES):
        nd = results[d]["outnd"].astype(np.float64)   # [128, NJC*CW]
        for k in range(NJC):
            blk = nd[:, k * CW:(k + 1) * CW]          # [128, CW]
            blocks.append(blk[:, 0:C_OUT] / blk[:, C_OUT + 1:C_OUT + 2])
    out = np.concatenate(blocks, axis=0) + bias.astype(np.float64)[None, :]
    return out.astype(np.float32)


def kernel(x, adj, W, att_src, att_dst, bias):
    nc = build_nc()
    in_maps = make_in_maps(x, adj, W, att_src, att_dst)
    res = run_bass_kernel_spmd(nc, in_maps, list(range(NCORES)))
    kernel._last_result = res
    return postprocess(res.results, bias)
